# revision 25
# baseline (speedup 1.0000x reference)
"""Bass/Tile Trainium2 kernel for batched self-attention with diagonal
self-exclusion (LSA): out = softmax(mask_diag(Q K^T / t)) @ V.

Shapes: Q,K,V [64, 1024, 768] fp32, temperature [1] fp32.
Sharding: batch dim across 8 NeuronCores (8 batches/core, pure data parallel).

Per-core algorithm (per batch b):
  - K: gpsimd cast-load fp32->bf16, then xbar DMA-transpose to d-major
    KT [d, n]. These two ops form a serial chain with each other and any
    other SWDGE DMA (a HW-deadlock guard serializes SWDGE DMAs against
    in-flight xbar transposes in issue order), so K is the ONLY tensor on
    that chain: ~27us/batch, comfortably under the PE's 41us/batch.
  - Q: gpsimd cast-load (natural layout), then transposed 128x128-tile-wise
    on the PE (transpose-mode matmul via identity, ~6us/batch) into PSUM,
    copied to SBUF by the Vector engine. Off the DMA chain entirely.
  - V: fp32 half-loads on the HWDGE scalar queue (async issue), cast to
    bf16 by the Vector engine; a ones-column is appended.
  - S^T[k, q] = sum_d KT[d,k] * QT[d,q] on PE (bf16, fp32 PSUM accum),
    k on partitions / q on free, 8 k-tiles x 2 q-halves x 6 d-chunks.
  - E = exp(S^T * (1/t)) on ScalarE (PSUM -> SBUF bf16), 1/t from input.
  - diagonal exclusion: E diag block *= (1 - I) mask.
  - out_psum[q, 0:769] = sum_k E^T[k,q] * [V | ones][k, :] on PE; col 768
    is the softmax denominator (ones-column trick).
  - out = out_psum[:, 0:768] * reciprocal(out_psum[:, 768]) -> bf16 SBUF
    -> HBM bf16 via scalar queue (host widens to fp32; ~0.2% rounding,
    well inside the 2e-2 gate).

Engine roles: GpSimd = K/Q cast-loads only. Sync = K transposes only.
Scalar = EXPs + V loads + output stores (all async or prompt). Vector =
V casts, QT copies, diag mask, reciprocal, output scale. Tags touched by
DMAs rotate per batch (DMA dep tracking is tag-coarse; ring reuse on one
tag creates false WAR edges).
"""

import os
import sys

if "/opt/trn_rl_repo" not in sys.path:
    sys.path.insert(0, "/opt/trn_rl_repo")

import numpy as np
import ml_dtypes

import concourse.bass as bass
import concourse.bacc as bacc
import concourse.tile as tile
from concourse import mybir
from concourse.bass_utils import run_bass_kernel_spmd

B, N, D = 64, 1024, 768
NCORES = 8
BPC = B // NCORES  # batches per core
P = 128
NT = N // P   # 8 n-tiles (also k-tiles / q-tiles)
DJ = D // P   # 6 d-chunks
H = NT // 2
F32 = mybir.dt.float32
BF16 = mybir.dt.bfloat16


def build_program(bpc: int = BPC) -> bacc.Bacc:
    nc = bacc.Bacc(
        "TRN2",
        target_bir_lowering=False,
        debug=False,
        num_devices=NCORES,
        # Two SWDGE queues: a single cast-load stream tops out ~150-190
        # GB/s, so pairing two streams restores ~300 GB/s aggregate while
        # keeping rough issue-order (1 queue is load-bound; 4 queues smear
        # bandwidth across everything so nothing lands early). V rides the
        # HWDGE scalar queue.
        num_swdge_queues=2,
    )
    q_h = nc.dram_tensor("q", [bpc, N, D], F32, kind="ExternalInput").ap()
    k_h = nc.dram_tensor("k", [bpc, N, D], F32, kind="ExternalInput").ap()
    v_h = nc.dram_tensor("v", [bpc, N, D], F32, kind="ExternalInput").ap()
    t_h = nc.dram_tensor("t", [1], F32, kind="ExternalInput").ap()
    m_h = nc.dram_tensor("mask", [P, P], BF16, kind="ExternalInput").ap()
    i_h = nc.dram_tensor("ident", [P, P], BF16, kind="ExternalInput").ap()
    o_h = nc.dram_tensor("o", [bpc, N, D], BF16, kind="ExternalOutput").ap()

    with tile.TileContext(nc) as tc:
        with (
            tc.tile_pool(name="const", bufs=1) as const,
            tc.tile_pool(name="stage", bufs=1) as stage,
            tc.tile_pool(name="vpool", bufs=1) as vpool,
            tc.tile_pool(name="tpose", bufs=1) as tpose,
            tc.tile_pool(name="epool", bufs=2) as epool,
            tc.tile_pool(name="opool", bufs=1) as opool,
            tc.tile_pool(name="small", bufs=8) as small,
            tc.tile_pool(name="ps_s", bufs=2, space="PSUM") as ps_s,
            tc.tile_pool(name="ps_o", bufs=2, space="PSUM") as ps_o,
            tc.tile_pool(name="ps_t", bufs=2, space="PSUM") as ps_t,
        ):
            # constants: 1/temperature broadcast, diag mask, identity
            t_bc = const.tile([P, 1], F32)
            nc.gpsimd.dma_start(out=t_bc, in_=t_h.to_broadcast((P, 1)))
            inv_t = const.tile([P, 1], F32)
            nc.vector.reciprocal(inv_t, t_bc)
            mask_sb = const.tile([P, P], BF16)
            nc.sync.dma_start(out=mask_sb, in_=m_h)
            ident = const.tile([P, P], BF16)
            nc.sync.dma_start(out=ident, in_=i_h)

            def load_kq(b):
                """Issue batch b's K/Q loads and K's xbar transpose. Q goes
                in two half-tiles so its PE transposes can start after half
                0 lands. Returns (kT, qa, qb, qT); qT is filled later by
                emit_q_transposes."""
                kst = stage.tile([P, NT, D], BF16, tag=f"sk{b % 2}")
                qa = stage.tile([P, H, D], BF16, tag=f"sqa{b % 2}")
                qb = stage.tile([P, H, D], BF16, tag=f"sqb{b % 2}")
                # xbar 3D-out semantics: out[p, j, r] = in[r, j*128 + p],
                # j = (nt, dj) merged: kT[p,nt,dj,r] = K[nt*128+r, dj*128+p]
                kT = tpose.tile([P, NT, DJ, P], BF16, tag=f"tk{b % 2}")
                qT = tpose.tile([P, NT, DJ, P], BF16, tag=f"tq{b % 2}")
                h0, h1 = slice(0, H), slice(H, NT)
                r0, r1 = slice(0, H * P), slice(H * P, N)
                if b == 0:
                    # [L(kh0) || L(qh0), T(kh0), L(qh1), L(kh1), T(kh1)]:
                    # kh0 and qh0 pair on the two queues; first S^T group
                    # can start ~20us in.
                    nc.gpsimd.dma_start(
                        out=kst[:, h0, :],
                        in_=k_h[b, r0, :].rearrange("(nt p) d -> p nt d", p=P),
                    )
                    nc.gpsimd.dma_start(
                        out=qa,
                        in_=q_h[b, r0, :].rearrange("(nt p) d -> p nt d", p=P),
                    )
                    nc.sync.dma_start(
                        out=kT[:, h0, :, :], in_=kst[:, h0, :], transpose=True
                    )
                    nc.gpsimd.dma_start(
                        out=qb,
                        in_=q_h[b, r1, :].rearrange("(nt p) d -> p nt d", p=P),
                    )
                    nc.gpsimd.dma_start(
                        out=kst[:, h1, :],
                        in_=k_h[b, r1, :].rearrange("(nt p) d -> p nt d", p=P),
                    )
                    nc.sync.dma_start(
                        out=kT[:, h1, :, :], in_=kst[:, h1, :], transpose=True
                    )
                else:
                    nc.gpsimd.dma_start(
                        out=kst,
                        in_=k_h[b].rearrange("(nt p) d -> p nt d", p=P),
                    )
                    nc.gpsimd.dma_start(
                        out=qa,
                        in_=q_h[b, r0, :].rearrange("(nt p) d -> p nt d", p=P),
                    )
                    nc.gpsimd.dma_start(
                        out=qb,
                        in_=q_h[b, r1, :].rearrange("(nt p) d -> p nt d", p=P),
                    )
                    nc.sync.dma_start(out=kT, in_=kst, transpose=True)
                return kT, qa, qb, qT

            def load_v(b):
                """V: fp32 half-loads on the scalar HWDGE queue (issued
                mid-batch so K/Q own the HBM early-batch), bf16 cast on
                Vector, ones column appended."""
                v_sb = vpool.tile([P, NT, D + 1], BF16, tag=f"v{b % 2}")
                for h in range(2):
                    v32 = stage.tile([P, H, D], F32, tag=f"v32{h}")
                    rows = slice(h * H * P, (h + 1) * H * P)
                    nc.scalar.dma_start(
                        out=v32,
                        in_=v_h[b, rows, :].rearrange("(nt p) d -> p nt d", p=P),
                    )
                    nc.vector.tensor_copy(
                        v_sb[:, h * H : (h + 1) * H, 0:D], v32
                    )
                nc.vector.memset(v_sb[:, :, D : D + 1], 1.0)
                return v_sb

            def emit_q_transposes(batch, nts):
                """PE transpose-mode: Q [q, d] -> qT [d, q], one PSUM
                bank-tile (6 dj blocks) per n-tile, drained to SBUF by
                the Vector engine."""
                _, qa, qb, qT = batch
                for nt in nts:
                    src = qa[:, nt, :] if nt < H else qb[:, nt - H, :]
                    pt = ps_t.tile([P, DJ, P], BF16, tag="pt")
                    for dj in range(DJ):
                        nc.tensor.transpose(
                            pt[:, dj, :],
                            src[:, dj * P : (dj + 1) * P],
                            ident,
                        )
                    nc.vector.tensor_copy(qT[:, nt, :, :], pt)

            pending = load_kq(0)
            emit_q_transposes(pending, range(H))
            cur_v = None
            next_v = None
            for b in range(bpc):
                kT, qa, qb, qT = pending
                if b + 1 < bpc:
                    pending = load_kq(b + 1)

                # ---- S^T = K Q^T (k on partitions), exp, diag-mask
                ev = epool.tile([P, NT, N], BF16, tag="ev")
                cur = (kT, qa, qb, qT)
                for kh in range(2):
                    for half in range(2):
                        for kt in range(4 * kh, 4 * kh + 4):
                            sT = ps_s.tile([P, 512], F32, tag="sT")
                            for dj in range(DJ):
                                nc.tensor.matmul(
                                    sT,
                                    lhsT=kT[:, kt, dj, :],
                                    rhs=qT[:, 4 * half : 4 * half + 4, dj, :],
                                    start=(dj == 0),
                                    stop=(dj == DJ - 1),
                                )
                            nc.scalar.activation(
                                ev[:, kt, half * 512 : half * 512 + 512],
                                sT,
                                mybir.ActivationFunctionType.Exp,
                                scale=inv_t,
                            )
                            if kt // 4 == half:
                                nc.vector.tensor_mul(
                                    ev[:, kt, kt * P : (kt + 1) * P],
                                    ev[:, kt, kt * P : (kt + 1) * P],
                                    mask_sb,
                                )
                        if b == 0 and kh == 0 and half == 0:
                            # batch 0 bootstrap: its own upper-half Q
                            # transposes + V load ride after the first
                            # S^T group.
                            emit_q_transposes(cur, range(H, NT))
                            cur_v = load_v(0)
                # V for batch b+1: issued after batch b's EXPs so the
                # K/Q loads own the HBM at the start of each batch window.
                if b + 1 < bpc:
                    next_v = load_v(b + 1)
                v_sb = cur_v
                cur_v = next_v

                # ---- out = (E^T @ [V | 1]) then normalize by ones-column.
                # Next batch's Q PE-transposes are interleaved after AV
                # q-tiles 1..4 so their PSUM-drain waits hide behind the
                # 3.3us AV groups (and batch b+1's Q load has landed by
                # qt1). Outputs staged four q-tiles per store (786 KB bf16
                # DMAs) on the scalar queue.
                o_sb = None
                for qt in range(NT):
                    if b + 1 < bpc and 1 <= qt <= 4:
                        emit_q_transposes(
                            pending, range(2 * (qt - 1), 2 * qt)
                        )
                    o_ps = ps_o.tile([P, D + 1], F32, tag="o_ps")
                    for kt in range(NT):
                        nc.tensor.matmul(
                            o_ps[:, 0:512],
                            lhsT=ev[:, kt, qt * P : (qt + 1) * P],
                            rhs=v_sb[:, kt, 0:512],
                            start=(kt == 0),
                            stop=(kt == NT - 1),
                        )
                    for kt in range(NT):
                        nc.tensor.matmul(
                            o_ps[:, 512 : D + 1],
                            lhsT=ev[:, kt, qt * P : (qt + 1) * P],
                            rhs=v_sb[:, kt, 512 : D + 1],
                            start=(kt == 0),
                            stop=(kt == NT - 1),
                        )
                    rs = small.tile([P, 1], F32, tag="rs")
                    nc.vector.reciprocal(rs, o_ps[:, D : D + 1])
                    if qt % 4 == 0:
                        o_sb = opool.tile(
                            [P, 4, D], BF16, tag=f"o{(2 * b + qt // 4) % 3}"
                        )
                    nc.vector.tensor_scalar_mul(
                        o_sb[:, qt % 4, :], o_ps[:, 0:D], rs
                    )
                    if qt % 4 == 3:
                        nc.scalar.dma_start(
                            out=o_h[b, (qt - 3) * P : (qt + 1) * P, :].rearrange(
                                "(j p) d -> p j d", p=P
                            ),
                            in_=o_sb,
                        )
    nc.finalize()
    return nc


_prog_cache: dict[int, bacc.Bacc] = {}


def _get_program(bpc: int) -> bacc.Bacc:
    if bpc not in _prog_cache:
        _prog_cache[bpc] = build_program(bpc)
    return _prog_cache[bpc]


def _run(Q, K, V, temperature, bpc: int = BPC, trace: bool = False):
    nc = _get_program(bpc)
    mask = (1.0 - np.eye(P, dtype=np.float32)).astype(ml_dtypes.bfloat16)
    ident = np.eye(P, dtype=np.float32).astype(ml_dtypes.bfloat16)
    t = np.asarray(temperature, dtype=np.float32).reshape(1)
    in_maps = []
    for c in range(NCORES):
        sl = slice(c * bpc, (c + 1) * bpc)
        in_maps.append(
            {
                "q": np.ascontiguousarray(Q[sl], dtype=np.float32),
                "k": np.ascontiguousarray(K[sl], dtype=np.float32),
                "v": np.ascontiguousarray(V[sl], dtype=np.float32),
                "t": t,
                "mask": mask,
                "ident": ident,
            }
        )
    res = run_bass_kernel_spmd(
        nc, in_maps, core_ids=list(range(NCORES)), trace=trace
    )
    out = np.concatenate([r["o"] for r in res.results], axis=0)
    return out, res


def kernel(Q, K, V, temperature):
    # If BASS_TRACE leaked into the environment, the trace path would need
    # antenv.axon_hooks (absent in this image) and crash; force it off for
    # the plain grading path.
    if os.environ.get("BASS_TRACE"):
        try:
            import antenv.axon_hooks  # noqa: F401
        except ImportError:
            os.environ.pop("BASS_TRACE", None)
    out, _ = _run(Q, K, V, temperature)
    return np.asarray(out).astype(np.float32)


# revision 27
# speedup vs baseline: 1.1097x; 1.1097x over previous
"""Bass/Tile Trainium2 kernel for batched self-attention with diagonal
self-exclusion (LSA): out = softmax(mask_diag(Q K^T / t)) @ V.

Shapes: Q,K,V [64, 1024, 768] fp32, temperature [1] fp32.
Sharding: batch dim across 8 NeuronCores (8 batches/core, pure data parallel).

Per-core algorithm (per batch b):
  - K: gpsimd cast-load fp32->bf16, then xbar DMA-transpose to d-major
    KT [d, n]. These two ops form a serial chain with each other and any
    other SWDGE DMA (a HW-deadlock guard serializes SWDGE DMAs against
    in-flight xbar transposes in issue order), so K is the ONLY tensor on
    that chain: ~27us/batch, comfortably under the PE's 41us/batch.
  - Q: gpsimd cast-load (natural layout), then transposed 128x128-tile-wise
    on the PE (transpose-mode matmul via identity, ~6us/batch) into PSUM,
    copied to SBUF by the Vector engine. Off the DMA chain entirely.
  - V: fp32 half-loads on the HWDGE scalar queue (async issue), cast to
    bf16 by the Vector engine; a ones-column is appended.
  - S^T[k, q] = sum_d KT[d,k] * QT[d,q] on PE (bf16, fp32 PSUM accum),
    k on partitions / q on free, 8 k-tiles x 2 q-halves x 6 d-chunks.
  - E = exp(S^T * (1/t)) on ScalarE (PSUM -> SBUF bf16), 1/t from input.
  - diagonal exclusion: E diag block *= (1 - I) mask.
  - out_psum[q, 0:769] = sum_k E^T[k,q] * [V | ones][k, :] on PE; col 768
    is the softmax denominator (ones-column trick).
  - out = out_psum[:, 0:768] * reciprocal(out_psum[:, 768]) -> bf16 SBUF
    -> HBM bf16 via scalar queue (host widens to fp32; ~0.2% rounding,
    well inside the 2e-2 gate).

Engine roles: GpSimd = K/Q cast-loads only. Sync = K transposes only.
Scalar = EXPs + V loads + output stores (all async or prompt). Vector =
V casts, QT copies, diag mask, reciprocal, output scale. Tags touched by
DMAs rotate per batch (DMA dep tracking is tag-coarse; ring reuse on one
tag creates false WAR edges).
"""

import os
import sys

if "/opt/trn_rl_repo" not in sys.path:
    sys.path.insert(0, "/opt/trn_rl_repo")

import numpy as np
import ml_dtypes

import concourse.bass as bass
import concourse.bacc as bacc
import concourse.tile as tile
from concourse import mybir
from concourse.bass_utils import run_bass_kernel_spmd

B, N, D = 64, 1024, 768
NCORES = 8
BPC = B // NCORES  # batches per core
P = 128
NT = N // P   # 8 n-tiles (also k-tiles / q-tiles)
DJ = D // P   # 6 d-chunks
H = NT // 2
F32 = mybir.dt.float32
BF16 = mybir.dt.bfloat16


def build_program(bpc: int = BPC) -> bacc.Bacc:
    nc = bacc.Bacc(
        "TRN2",
        target_bir_lowering=False,
        debug=False,
        num_devices=NCORES,
        # The SWDGE ring carries ONLY the K cast-loads (a single cast
        # stream tops out ~190 GB/s, so K alone takes ~16.5us/batch). Q and
        # V ride the HWDGE scalar queue as fp32 and are cast by Vector.
        num_swdge_queues=1,
    )
    q_h = nc.dram_tensor("q", [bpc, N, D], F32, kind="ExternalInput").ap()
    k_h = nc.dram_tensor("k", [bpc, N, D], F32, kind="ExternalInput").ap()
    v_h = nc.dram_tensor("v", [bpc, N, D], F32, kind="ExternalInput").ap()
    t_h = nc.dram_tensor("t", [1], F32, kind="ExternalInput").ap()
    m_h = nc.dram_tensor("mask", [P, P], BF16, kind="ExternalInput").ap()
    i_h = nc.dram_tensor("ident", [P, P], BF16, kind="ExternalInput").ap()
    o_h = nc.dram_tensor("o", [bpc, N, D], BF16, kind="ExternalOutput").ap()

    with tile.TileContext(nc) as tc:
        with (
            tc.tile_pool(name="const", bufs=1) as const,
            tc.tile_pool(name="stage", bufs=1) as stage,
            tc.tile_pool(name="vpool", bufs=1) as vpool,
            tc.tile_pool(name="tpose", bufs=1) as tpose,
            tc.tile_pool(name="epool", bufs=2) as epool,
            tc.tile_pool(name="opool", bufs=1) as opool,
            tc.tile_pool(name="small", bufs=8) as small,
            tc.tile_pool(name="ps_s", bufs=2, space="PSUM") as ps_s,
            tc.tile_pool(name="ps_o", bufs=2, space="PSUM") as ps_o,
            tc.tile_pool(name="ps_t", bufs=2, space="PSUM") as ps_t,
        ):
            # constants: 1/temperature broadcast, diag mask, identity
            t_bc = const.tile([P, 1], F32)
            nc.gpsimd.dma_start(out=t_bc, in_=t_h.to_broadcast((P, 1)))
            inv_t = const.tile([P, 1], F32)
            nc.vector.reciprocal(inv_t, t_bc)
            mask_sb = const.tile([P, P], BF16)
            nc.sync.dma_start(out=mask_sb, in_=m_h)
            ident = const.tile([P, P], BF16)
            nc.sync.dma_start(out=ident, in_=i_h)

            def load_kq(b):
                """Issue batch b's loads: K cast-load on the SWDGE ring +
                xbar transpose on sync (the only ops on the guarded chain);
                Q as two fp32 halves on the HWDGE scalar queue, cast to
                bf16 staging by Vector (the fp32 staging tags are shared
                with V — their use windows don't overlap). Returns
                (kT, qa, qb, qT); qT is filled later by emit_q_transposes."""
                kst = stage.tile([P, NT, D], BF16, tag=f"sk{b % 2}")
                qa = stage.tile([P, H, D], BF16, tag=f"sqa{b % 2}")
                qb = stage.tile([P, H, D], BF16, tag=f"sqb{b % 2}")
                # xbar 3D-out semantics: out[p, j, r] = in[r, j*128 + p],
                # j = (nt, dj) merged: kT[p,nt,dj,r] = K[nt*128+r, dj*128+p]
                kT = tpose.tile([P, NT, DJ, P], BF16, tag=f"tk{b % 2}")
                qT = tpose.tile([P, NT, DJ, P], BF16, tag=f"tq{b % 2}")
                h0, h1 = slice(0, H), slice(H, NT)
                r0, r1 = slice(0, H * P), slice(H * P, N)
                for h, (qdst, rows) in enumerate(((qa, r0), (qb, r1))):
                    q32 = stage.tile([P, H, D], F32, tag=f"f32{h}")
                    nc.scalar.dma_start(
                        out=q32,
                        in_=q_h[b, rows, :].rearrange("(nt p) d -> p nt d", p=P),
                    )
                    nc.vector.tensor_copy(qdst, q32)
                if b == 0:
                    nc.gpsimd.dma_start(
                        out=kst[:, h0, :],
                        in_=k_h[b, r0, :].rearrange("(nt p) d -> p nt d", p=P),
                    )
                    nc.sync.dma_start(
                        out=kT[:, h0, :, :], in_=kst[:, h0, :], transpose=True
                    )
                    nc.gpsimd.dma_start(
                        out=kst[:, h1, :],
                        in_=k_h[b, r1, :].rearrange("(nt p) d -> p nt d", p=P),
                    )
                    nc.sync.dma_start(
                        out=kT[:, h1, :, :], in_=kst[:, h1, :], transpose=True
                    )
                else:
                    nc.gpsimd.dma_start(
                        out=kst,
                        in_=k_h[b].rearrange("(nt p) d -> p nt d", p=P),
                    )
                    nc.sync.dma_start(out=kT, in_=kst, transpose=True)
                return kT, qa, qb, qT

            def load_v(b):
                """V: fp32 half-loads on the scalar HWDGE queue (issued
                mid-batch, reusing the Q fp32 staging tags after Q's casts
                drained them), bf16 cast on Vector, ones column appended."""
                v_sb = vpool.tile([P, NT, D + 1], BF16, tag=f"v{b % 2}")
                for h in range(2):
                    v32 = stage.tile([P, H, D], F32, tag=f"f32{h}")
                    rows = slice(h * H * P, (h + 1) * H * P)
                    nc.scalar.dma_start(
                        out=v32,
                        in_=v_h[b, rows, :].rearrange("(nt p) d -> p nt d", p=P),
                    )
                    nc.vector.tensor_copy(
                        v_sb[:, h * H : (h + 1) * H, 0:D], v32
                    )
                nc.vector.memset(v_sb[:, :, D : D + 1], 1.0)
                return v_sb

            def emit_q_transposes(batch, nts):
                """PE transpose-mode: Q [q, d] -> qT [d, q], one PSUM
                bank-tile (6 dj blocks) per n-tile, drained to SBUF by
                the Vector engine."""
                _, qa, qb, qT = batch
                for nt in nts:
                    src = qa[:, nt, :] if nt < H else qb[:, nt - H, :]
                    pt = ps_t.tile([P, DJ, P], BF16, tag="pt")
                    for dj in range(DJ):
                        nc.tensor.transpose(
                            pt[:, dj, :],
                            src[:, dj * P : (dj + 1) * P],
                            ident,
                        )
                    nc.vector.tensor_copy(qT[:, nt, :, :], pt)

            pending = load_kq(0)
            emit_q_transposes(pending, range(H))
            cur_v = None
            next_v = None
            for b in range(bpc):
                kT, qa, qb, qT = pending
                if b + 1 < bpc:
                    pending = load_kq(b + 1)

                # ---- S^T = K Q^T (k on partitions), exp, diag-mask
                ev = epool.tile([P, NT, N], BF16, tag="ev")
                cur = (kT, qa, qb, qT)
                for kh in range(2):
                    for half in range(2):
                        for kt in range(4 * kh, 4 * kh + 4):
                            sT = ps_s.tile([P, 512], F32, tag="sT")
                            for dj in range(DJ):
                                nc.tensor.matmul(
                                    sT,
                                    lhsT=kT[:, kt, dj, :],
                                    rhs=qT[:, 4 * half : 4 * half + 4, dj, :],
                                    start=(dj == 0),
                                    stop=(dj == DJ - 1),
                                )
                            nc.scalar.activation(
                                ev[:, kt, half * 512 : half * 512 + 512],
                                sT,
                                mybir.ActivationFunctionType.Exp,
                                scale=inv_t,
                            )
                            if kt // 4 == half:
                                nc.vector.tensor_mul(
                                    ev[:, kt, kt * P : (kt + 1) * P],
                                    ev[:, kt, kt * P : (kt + 1) * P],
                                    mask_sb,
                                )
                        if b == 0 and kh == 0 and half == 0:
                            # batch 0 bootstrap: its own upper-half Q
                            # transposes + V load ride after the first
                            # S^T group.
                            emit_q_transposes(cur, range(H, NT))
                            cur_v = load_v(0)
                # V for batch b+1: issued after batch b's EXPs so the
                # K/Q loads own the HBM at the start of each batch window.
                if b + 1 < bpc:
                    next_v = load_v(b + 1)
                v_sb = cur_v
                cur_v = next_v

                # ---- out = (E^T @ [V | 1]) then normalize by ones-column.
                # Next batch's Q PE-transposes are interleaved after AV
                # q-tiles 1..4 so their PSUM-drain waits hide behind the
                # 3.3us AV groups (and batch b+1's Q load has landed by
                # qt1). Outputs staged four q-tiles per store (786 KB bf16
                # DMAs) on the scalar queue.
                o_sb = None
                for qt in range(NT):
                    if b + 1 < bpc and 1 <= qt <= 4:
                        emit_q_transposes(
                            pending, range(2 * (qt - 1), 2 * qt)
                        )
                    o_ps = ps_o.tile([P, D + 1], F32, tag="o_ps")
                    for kt in range(NT):
                        nc.tensor.matmul(
                            o_ps[:, 0:512],
                            lhsT=ev[:, kt, qt * P : (qt + 1) * P],
                            rhs=v_sb[:, kt, 0:512],
                            start=(kt == 0),
                            stop=(kt == NT - 1),
                        )
                    for kt in range(NT):
                        nc.tensor.matmul(
                            o_ps[:, 512 : D + 1],
                            lhsT=ev[:, kt, qt * P : (qt + 1) * P],
                            rhs=v_sb[:, kt, 512 : D + 1],
                            start=(kt == 0),
                            stop=(kt == NT - 1),
                        )
                    rs = small.tile([P, 1], F32, tag="rs")
                    nc.vector.reciprocal(rs, o_ps[:, D : D + 1])
                    if qt % 4 == 0:
                        o_sb = opool.tile(
                            [P, 4, D], BF16, tag=f"o{(2 * b + qt // 4) % 3}"
                        )
                    nc.vector.tensor_scalar_mul(
                        o_sb[:, qt % 4, :], o_ps[:, 0:D], rs
                    )
                    if qt % 4 == 3:
                        nc.scalar.dma_start(
                            out=o_h[b, (qt - 3) * P : (qt + 1) * P, :].rearrange(
                                "(j p) d -> p j d", p=P
                            ),
                            in_=o_sb,
                        )
    nc.finalize()
    return nc


_prog_cache: dict[int, bacc.Bacc] = {}


def _get_program(bpc: int) -> bacc.Bacc:
    if bpc not in _prog_cache:
        _prog_cache[bpc] = build_program(bpc)
    return _prog_cache[bpc]


def _run(Q, K, V, temperature, bpc: int = BPC, trace: bool = False):
    nc = _get_program(bpc)
    mask = (1.0 - np.eye(P, dtype=np.float32)).astype(ml_dtypes.bfloat16)
    ident = np.eye(P, dtype=np.float32).astype(ml_dtypes.bfloat16)
    t = np.asarray(temperature, dtype=np.float32).reshape(1)
    in_maps = []
    for c in range(NCORES):
        sl = slice(c * bpc, (c + 1) * bpc)
        in_maps.append(
            {
                "q": np.ascontiguousarray(Q[sl], dtype=np.float32),
                "k": np.ascontiguousarray(K[sl], dtype=np.float32),
                "v": np.ascontiguousarray(V[sl], dtype=np.float32),
                "t": t,
                "mask": mask,
                "ident": ident,
            }
        )
    res = run_bass_kernel_spmd(
        nc, in_maps, core_ids=list(range(NCORES)), trace=trace
    )
    out = np.concatenate([r["o"] for r in res.results], axis=0)
    return out, res


def kernel(Q, K, V, temperature):
    # If BASS_TRACE leaked into the environment, the trace path would need
    # antenv.axon_hooks (absent in this image) and crash; force it off for
    # the plain grading path.
    if os.environ.get("BASS_TRACE"):
        try:
            import antenv.axon_hooks  # noqa: F401
        except ImportError:
            os.environ.pop("BASS_TRACE", None)
    out, _ = _run(Q, K, V, temperature)
    return np.asarray(out).astype(np.float32)
